# revision 5
# baseline (speedup 1.0000x reference)
"""Causal single-head attention kernel for TRN2 (one batch element per core).

Computes: out = softmax(causal((X_q Wq + bq)(X_k Wk + bk)^T / sqrt(H))) (X_v Wv + bv)
Shapes per core: Q,K,V [S, E]; Wq/Wk/Wv [E, H]; bq/bk/bv [H]; out [S, H].

v2 design:
- All transposes done by the DMA XBAR (dma_start_transpose, bf16, issued on
  the Sync queue) -- the PE does only real matmuls (projections, scores, PV).
- Input DMAs (gpsimd, f32->bf16 cast) issued before constant setup, in
  consumption order; Q3 streamed before K3/V3 so the last attention chunk's
  off-diagonal tiles run while K3/V3 still stream.
- Softmax denominators: exp'd score tiles are DMA-transposed and reduced on
  the vector engine (free-axis) -- no ones-matmuls on the PE.
- One interleaved loop: project K/V/Q of chunk c, attention c, finalize c;
  the tile framework overlaps next-chunk DMA/transposes with attention.
"""

from contextlib import ExitStack

import numpy as np

import concourse.bacc as bacc
import concourse.bass as bass
import concourse.mybir as mybir
import concourse.tile as tile

F32 = mybir.dt.float32
BF16 = mybir.dt.bfloat16

CH = 512          # Sq chunk width (psum bank)
PT = 128          # partition tile


def build(S=2048, E=1024, H=128, n_cores=8):
    """Build + compile the Bacc kernel. Returns nc."""
    EC = E // PT              # E chunks (8)
    NCHUNK = S // CH          # Sq chunks (4)
    TPC = CH // PT            # S-tiles per chunk (4)
    scale = float(H) ** -0.5

    nc = bacc.Bacc("TRN2", target_bir_lowering=False, debug=False,
                   num_devices=n_cores)

    Qd = nc.declare_dram_parameter("Q", [S, E], F32, isOutput=False)
    Kd = nc.declare_dram_parameter("K", [S, E], F32, isOutput=False)
    Vd = nc.declare_dram_parameter("V", [S, E], F32, isOutput=False)
    Wqd = nc.declare_dram_parameter("Wq", [E, H], F32, isOutput=False)
    Wkd = nc.declare_dram_parameter("Wk", [E, H], F32, isOutput=False)
    Wvd = nc.declare_dram_parameter("Wv", [E, H], F32, isOutput=False)
    bqd = nc.declare_dram_parameter("bq", [H], F32, isOutput=False)
    bkd = nc.declare_dram_parameter("bk", [H], F32, isOutput=False)
    bvd = nc.declare_dram_parameter("bv", [H], F32, isOutput=False)
    outd = nc.declare_dram_parameter("out", [S, H], F32, isOutput=True)

    xd = {"q": Qd, "k": Kd, "v": Vd}

    with tile.TileContext(nc) as tc, ExitStack() as ctx:
        persist = ctx.enter_context(tc.tile_pool(name="persist", bufs=1))
        xn_p = ctx.enter_context(tc.tile_pool(name="xn", bufs=1))
        xt_p = ctx.enter_context(tc.tile_pool(name="xt", bufs=20))
        ew_p = ctx.enter_context(tc.tile_pool(name="ew", bufs=8))
        ewt_p = ctx.enter_context(tc.tile_pool(name="ewt", bufs=4))
        small_p = ctx.enter_context(tc.tile_pool(name="small", bufs=6))

        ps_mm = ctx.enter_context(tc.tile_pool(name="ps_mm", bufs=4, space="PSUM"))
        ps_outT = ctx.enter_context(tc.tile_pool(name="ps_outT", bufs=1, space="PSUM"))

        # ---- weights + biases first (small, land before first projection) ----
        wts = {}
        for nm, d in (("k", Wkd), ("v", Wvd), ("q", Wqd)):
            w = persist.tile([PT, EC, H], BF16, tag=f"w{nm}")
            nc.gpsimd.dma_start(out=w, in_=d[:].rearrange("(c p) h -> p c h", p=PT))
            wts[nm] = w
        bias = {}
        for nm, d in (("k", bkd), ("v", bvd), ("q", bqd)):
            b = persist.tile([H, 1], F32, tag=f"b{nm}")
            nc.gpsimd.dma_start(out=b, in_=d[:].unsqueeze(1))
            bias[nm] = b

        # ---- input stream: all 12 chunk DMAs issued up front (gpsimd casts) ----
        stream = [("k", 0), ("v", 0), ("q", 0),
                  ("k", 1), ("v", 1), ("q", 1),
                  ("k", 2), ("v", 2), ("q", 2),
                  ("q", 3), ("k", 3), ("v", 3)]
        xn = {}
        for nm, c in stream:
            t_ = xn_p.tile([PT, TPC, E], BF16, tag=f"xn_{nm}{c}")
            nc.gpsimd.dma_start(
                out=t_, in_=xd[nm][c * CH:(c + 1) * CH, :].rearrange(
                    "(t p) e -> p t e", p=PT))
            xn[(nm, c)] = t_

        # ---- constants (behind the DMA issues on gpsimd) ----
        masks = []
        for m in range(TPC):
            mk = persist.tile([PT, CH], BF16, tag=f"mask{m}")
            nc.gpsimd.memset(mk, 1.0)
            # keep (=1.0) where f - p - 128*m >= 0 else fill 0.0
            nc.gpsimd.affine_select(
                out=mk, in_=mk, compare_op=mybir.AluOpType.is_ge,
                fill=0.0, base=-PT * m, pattern=[[1, CH]], channel_multiplier=-1,
            )
            masks.append(mk)

        # persistent projected tensors
        qT = [persist.tile([H, CH], BF16, tag=f"qT{c}", name=f"qT{c}")
              for c in range(NCHUNK)]
        kT = [persist.tile([H, CH], BF16, tag=f"kT{c}", name=f"kT{c}")
              for c in range(NCHUNK)]
        vnat = [persist.tile([PT, H], BF16, tag=f"v{j}", name=f"v{j}")
                for j in range(S // PT)]

        # ---- helpers ----
        def project(nm, c):
            """xn[(nm,c)] -> qT[c]/kT[c]/vnat[4c..4c+3] via DMA transposes + PE."""
            xnt = xn[(nm, c)]
            w = wts[nm]
            b = bias[nm]
            xts = []
            for e in range(EC):
                xt = xt_p.tile([PT, CH], BF16, tag="xt")
                for t in range(TPC):
                    nc.sync.dma_start_transpose(
                        out=xt[:, t * PT:(t + 1) * PT],
                        in_=xnt[:, t, e * PT:(e + 1) * PT])
                xts.append(xt)
            pj = ps_mm.tile([H, CH], F32, tag="mm")
            for e in range(EC):
                nc.tensor.matmul(pj, w[:, e, :], xts[e],
                                 start=(e == 0), stop=(e == EC - 1))
            if nm == "q":
                nc.vector.tensor_scalar_add(qT[c], pj, b[:])
            elif nm == "k":
                nc.vector.tensor_scalar_add(kT[c], pj, b[:])
            else:
                vTb = small_p.tile([H, CH], BF16, tag="vTb")
                nc.vector.tensor_scalar_add(vTb, pj, b[:])
                for t in range(TPC):
                    nc.sync.dma_start_transpose(
                        out=vnat[c * TPC + t],
                        in_=vTb[:, t * PT:(t + 1) * PT])

        def attn_tiles(c, js, oT, acc, first):
            """Score/exp/PV for Sk tiles `js` of Sq chunk c."""
            nk = (c + 1) * TPC
            for j in js:
                wp = ps_mm.tile([PT, CH], F32, tag="mm")
                kc, kt = divmod(j, TPC)
                nc.tensor.matmul(wp, kT[kc][:, kt * PT:(kt + 1) * PT],
                                 qT[c], start=True, stop=True)
                ew = ew_p.tile([PT, CH], BF16, tag="ew")
                nc.scalar.activation(out=ew, in_=wp,
                                     func=mybir.ActivationFunctionType.Exp,
                                     scale=scale)
                m = j - c * TPC
                if m >= 0:
                    nc.vector.tensor_mul(ew, ew, masks[m])
                # PV accumulate on PE
                nc.tensor.matmul(oT, vnat[j][:], ew,
                                 start=(j == 0), stop=(j == nk - 1))
                # denominators: transpose ew, reduce on vector
                ewt = ewt_p.tile([PT, TPC, PT], BF16, tag="ewt")
                for t in range(TPC):
                    nc.sync.dma_start_transpose(
                        out=ewt[:, t, :], in_=ew[:, t * PT:(t + 1) * PT])
                if j == first:
                    nc.vector.tensor_reduce(
                        out=acc, in_=ewt, axis=mybir.AxisListType.X,
                        op=mybir.AluOpType.add)
                else:
                    sj = small_p.tile([PT, TPC], F32, tag="sj")
                    nc.vector.tensor_reduce(
                        out=sj, in_=ewt, axis=mybir.AxisListType.X,
                        op=mybir.AluOpType.add)
                    nc.vector.tensor_add(acc, acc, sj)

        def finalize(c, oT, acc):
            recip = small_p.tile([PT, TPC], F32, tag="recip")
            nc.vector.reciprocal(recip, acc)
            oTb = small_p.tile([H, CH], BF16, tag="oTb")
            nc.vector.tensor_copy(oTb, oT)
            otp = small_p.tile([PT, TPC, PT], BF16, tag="otp")
            for t in range(TPC):
                nc.sync.dma_start_transpose(
                    out=otp[:, t, :], in_=oTb[:, t * PT:(t + 1) * PT])
            for t in range(TPC):
                ob = small_p.tile([PT, H], F32, tag="ob")
                nc.vector.tensor_scalar_mul(ob, otp[:, t, :], recip[:, t:t + 1])
                nc.gpsimd.dma_start(
                    out=outd[c * CH + t * PT: c * CH + (t + 1) * PT, :], in_=ob)

        # ---- interleaved main loop ----
        for c in range(3):
            project("k", c)
            project("v", c)
            project("q", c)
            oT = ps_outT.tile([H, CH], F32, tag="outT")
            acc = small_p.tile([PT, TPC], F32, tag="acc")
            attn_tiles(c, range((c + 1) * TPC), oT, acc, first=0)
            finalize(c, oT, acc)

        # chunk 3: Q first, off-diagonal attention, then K3/V3, diagonal tiles
        project("q", 3)
        oT = ps_outT.tile([H, CH], F32, tag="outT")
        acc = small_p.tile([PT, TPC], F32, tag="acc")
        attn_tiles(3, range(12), oT, acc, first=0)
        project("k", 3)
        project("v", 3)
        attn_tiles(3, range(12, 16), oT, acc, first=0)
        finalize(3, oT, acc)

    nc.compile()
    return nc


_NC_CACHE = {}


def _get_nc():
    if "nc" not in _NC_CACHE:
        _NC_CACHE["nc"] = build(S=2048, E=1024, H=128, n_cores=8)
    return _NC_CACHE["nc"]


def kernel(Q, K, V, mask=None, Wq=None, bq=None, Wk=None, bk=None,
           Wv=None, bv=None, **_):
    """Full-input entry point: Q/K/V [8, 2048, 1024] fp32 -> out [8, 2048, 128].

    Data-parallel over batch: core i computes batch element i. The causal
    mask input is ignored (causality is hardcoded in the kernel structure).
    """
    from concourse.bass_utils import run_bass_kernel_spmd

    B = Q.shape[0]
    nc = _get_nc()
    f32 = np.float32
    in_maps = []
    for i in range(B):
        in_maps.append({
            "Q": np.ascontiguousarray(Q[i], dtype=f32),
            "K": np.ascontiguousarray(K[i], dtype=f32),
            "V": np.ascontiguousarray(V[i], dtype=f32),
            "Wq": np.ascontiguousarray(Wq, dtype=f32),
            "Wk": np.ascontiguousarray(Wk, dtype=f32),
            "Wv": np.ascontiguousarray(Wv, dtype=f32),
            "bq": np.ascontiguousarray(bq, dtype=f32),
            "bk": np.ascontiguousarray(bk, dtype=f32),
            "bv": np.ascontiguousarray(bv, dtype=f32),
        })
    r = run_bass_kernel_spmd(nc, in_maps, list(range(B)))
    return np.stack([r.results[i]["out"] for i in range(B)]).astype(np.float32)
